# revision 32
# baseline (speedup 1.0000x reference)
"""Causal self-attention (B=4, T=2048, D=1024, H=16) on 8 trn2 NeuronCores.

Sharding: Megatron-style tensor parallel over heads (TP=2) x data parallel
over batch (DP=4). Core c handles batch c//2 and head-group c%2 (8 heads).
Each core computes its QKV projection slice, causal attention for its 8
heads, and a partial output projection; the host sums the two TP partials
per batch and adds b_proj.

v2 over the v1 baseline:
- S (scores) matmuls contract K=64 per head and run as two concurrent
  row-tiled matmuls (tile_position rows 0-63 / 64-127) instead of
  zero-padded K=128 -> ~2x faster score phase.
- k bias dropped entirely: softmax is invariant to a per-query additive
  constant, and (q+bq).(k+bk) - (q+bq).k depends only on the query.
- causal mask applied as a fp16 0/1 multiply on es AFTER exp (DVE 2x mode)
  instead of a -1e30 fp32 add on PSUM before exp.
- normalize path reads PSUM directly (reciprocal of the ones-row, gpsimd
  partition-broadcast, single multiply) - no staging copies.
- software pipelining: QKV/V projections run at 512-token granularity and
  are interleaved into the (exp-bound) attention chunks so the PE always
  has independent matmul work while the scalar engine crunches exps.
- HAM warmup: a burst of dummy matmuls at t=0 unparks the PE clock gate
  while the input DMAs stream in.

All matmuls run in fp16 (fp32 PSUM accumulation); softmax runs in fp32 on
the scalar engine (exp) / DVE (reciprocal).
"""
import sys

sys.path.insert(0, "/opt/trn_rl_repo")

import numpy as np
import ml_dtypes

import concourse.bass as bass
import concourse.tile as tile
from concourse import bacc, mybir
from concourse.bass_utils import run_bass_kernel_spmd

B, T, D, H = 4, 2048, 1024, 16
HD = 64            # head dim
HL = 8             # heads per core (TP=2)
DL = HL * HD       # 512 local qkv width
KCH = D // 128     # 8 contraction chunks for QKV
TCH = T // 128     # 16 T chunks of 128
TB = T // 512      # 4 T blocks of 512
F16 = mybir.dt.float16
F32 = mybir.dt.float32
F8 = mybir.dt.float8e4
DR = mybir.MatmulPerfMode.DoubleRow

_cache = {}


def _build():
    nc = bacc.Bacc("TRN2", target_bir_lowering=False, num_devices=8)

    # inputs are host-prepped into fp8 pair layouts for DoubleRow matmuls
    # (contraction runs over chunk PAIRS: lhsT [p, 2, M] j-major, rhs
    # [p, N, 2] j-adjacent), and load with few big DMA descriptors (issue
    # cost on the sequencers is ~600-800ns per dma_start).
    xT8i = nc.dram_tensor("xT8i", [128, KCH // 2, T, 2], F8,
                          kind="ExternalInput")
    xT = nc.dram_tensor("xT", [128, KCH, T], F16, kind="ExternalInput")
    w8qk = nc.dram_tensor("w8qk", [128, KCH // 2, 2, 2 * DL], F8,
                          kind="ExternalInput")
    bq = nc.dram_tensor("bq", [128, DL // 128], F32, kind="ExternalInput")
    wv = nc.dram_tensor("wv", [128, KCH, DL], F16, kind="ExternalInput")
    bv = nc.dram_tensor("bv", [1, DL], F32, kind="ExternalInput")
    wp = nc.dram_tensor("wp", [128, DL // 128, D], F16, kind="ExternalInput")
    tri = nc.dram_tensor("tri", [128, 128], F16, kind="ExternalInput")
    out = nc.dram_tensor("out", [T, D], F32, kind="ExternalOutput")

    with tile.TileContext(nc) as tc:
        with (
            tc.tile_pool(name="const", bufs=1) as const,
            tc.tile_pool(name="acts", bufs=1) as acts,
            tc.tile_pool(name="work", bufs=4) as work,
            tc.tile_pool(name="work8", bufs=3) as work8,
            tc.tile_pool(name="small", bufs=4) as small,
            tc.tile_pool(name="outp", bufs=3) as outp,
            tc.tile_pool(name="psS", bufs=2, space="PSUM") as psS,
            tc.tile_pool(name="psP", bufs=2, space="PSUM") as psP,
            tc.tile_pool(name="psy", bufs=2, space="PSUM") as psy,
        ):
            # ---- HAM warmup: dummy matmuls while DMAs stream ----
            wtile = const.tile([128, 512], F16, name="warm", tag="warm")
            nc.vector.memset(wtile, 0.0)
            for i in range(34):
                ps_w = psP.tile([128, 512], F32, name="psP", tag="psP")
                nc.tensor.matmul(ps_w, wtile[:, 0:128], wtile,
                                 start=True, stop=True)

            # ---- resident inputs ----
            # gpsimd DMA queue: tiny tensors, wqk (q cols then k cols), wp
            bq_sb = const.tile([128, DL // 128], F32)
            nc.gpsimd.dma_start(out=bq_sb, in_=bq[:, :])
            bv_sb = const.tile([1, DL], F32)
            nc.gpsimd.dma_start(out=bv_sb, in_=bv[:, :])
            tri_sb = const.tile([128, 128], F16)
            nc.gpsimd.dma_start(out=tri_sb, in_=tri[:, :])
            w8qk_t = const.tile([128, (KCH // 2) * 2 * 2 * DL], F8,
                                name="w8qk", tag="w8qk")
            w8qk_r = w8qk_t.rearrange("p (u j m) -> p u j m", u=KCH // 2,
                                      j=2)
            nc.gpsimd.dma_start(out=w8qk_r[:, :, :, 0:DL],
                                in_=w8qk[:, :, :, 0:DL])
            nc.gpsimd.dma_start(out=w8qk_r[:, :, :, DL:2 * DL],
                                in_=w8qk[:, :, :, DL:2 * DL])
            wp_all = const.tile([128, DL // 128, D], F16, name="wp",
                                tag="wp")
            nc.gpsimd.dma_start(out=wp_all, in_=wp[:, :, :])
            # sync DMA queue: x block 0 (fp8 then fp16), wv, x blocks 1-3
            xT8i_t = const.tile([128, (KCH // 2) * T * 2], F8, name="xT8i",
                                tag="xT8i")
            xT8i_w = xT8i_t.rearrange("p (u t j) -> p u t j", u=KCH // 2,
                                      j=2)
            xT8i_r = xT8i_t.rearrange("p (u t j) -> p u j t", u=KCH // 2,
                                      j=2)
            xT_all = const.tile([128, KCH, T], F16, name="xT", tag="xT")
            nc.sync.dma_start(out=xT8i_w[:, :, 0:512, :],
                              in_=xT8i[:, :, 0:512, :])
            nc.sync.dma_start(out=xT_all[:, :, 0:512], in_=xT[:, :, 0:512])
            wv_all = const.tile([128, KCH, DL], F16, name="wv", tag="wv")
            nc.sync.dma_start(out=wv_all, in_=wv[:, :, :])
            nc.sync.dma_start(out=xT8i_w[:, :, 512:T, :],
                              in_=xT8i[:, :, 512:T, :])
            nc.sync.dma_start(out=xT_all[:, :, 512:T], in_=xT[:, :, 512:T])
            bvb_sb = const.tile([128, DL], F32)
            nc.gpsimd.partition_broadcast(bvb_sb, bv_sb)

            # ---- persistent activations ----
            qT_sb = [acts.tile([128, T], F16, name=f"qT{c}", tag=f"qT{c}")
                     for c in range(4)]
            # kTp[c]: partitions 0:64 = head 2c's k (64 dims), 64:128 = head
            # 2c+1's.  S matmuls contract K=64 per head, issued as two
            # concurrent row-tiled matmuls (rows 0-63 and 64-127).
            kTp_sb = [acts.tile([128, T], F16, name=f"kTp{c}", tag=f"kTp{c}")
                      for c in range(4)]
            vaug = [acts.tile([128, HL * (HD + 1)], F16, name=f"va{t}",
                              tag=f"va{t}") for t in range(TCH)]
            # fp8 copies of vaug for DoubleRow PV: pair u holds tiles
            # (2u, 2u+1) as [p, j(2), h(8), m(80; 65 used)] - the 80-slot
            # pad keeps the Ko step 16-byte aligned.
            w8 = [acts.tile([128, 2 * HL * 80], F8, name=f"w8{u}",
                            tag=f"w8{u}") for u in range(TCH // 2)]
            yT_sb = [acts.tile([128, T], F16, name=f"yT{c}", tag=f"yT{c}")
                     for c in range(4)]

            # ---------- emission helpers ----------
            def qkv_block(tb, cc):
                """q/k projection for 512-token block tb, output chunk cc
                (cc<4 -> qT[cc], cc>=4 -> kTp[cc-4]).  Emits 8 matmuls +
                one evacuation."""
                ps_w = psP.tile([128, 512], F32, name="psP", tag="psP")
                ts = slice(512 * tb, 512 * (tb + 1))
                for u in range(KCH // 2):
                    nc.tensor.matmul(
                        ps_w,
                        w8qk_r[:, u, :, 128 * cc:128 * (cc + 1)],
                        xT8i_r[:, u, :, ts],
                        start=(u == 0), stop=(u == KCH // 2 - 1),
                        perf_mode=DR,
                    )
                if cc < 4:
                    nc.vector.tensor_scalar_add(
                        out=qT_sb[cc][:, ts],
                        in0=ps_w,
                        scalar1=bq_sb[:, cc:cc + 1],
                    )
                else:
                    nc.vector.tensor_copy(kTp_sb[cc - 4][:, ts], ps_w)

            def v_block(t):
                """v (natural layout) + ones column for 128-token tile t."""
                ps_w = psP.tile([128, 512], F32, name="psP", tag="psP")
                for k in range(KCH):
                    nc.tensor.matmul(
                        ps_w,
                        xT_all[:, k, 128 * t:128 * (t + 1)],
                        wv_all[:, k, :],
                        start=(k == 0), stop=(k == KCH - 1),
                    )
                va = vaug[t]
                va3 = va.rearrange("p (h c) -> p h c", c=HD + 1)
                nc.vector.tensor_add(
                    va3[:, :, 0:HD],
                    ps_w.rearrange("p (h d) -> p h d", d=HD),
                    bvb_sb.rearrange("p (h d) -> p h d", d=HD),
                )
                nc.gpsimd.memset(va3[:, :, HD], 1.0)

            def w8_fill(t):
                """fp8 copy of vaug[t] into its DoubleRow pair slot."""
                u, j = t // 2, t % 2
                dst = w8[u].rearrange("p (j h m) -> p j h m", j=2, m=80)
                src = vaug[t].rearrange("p (h c) -> p h c", c=HD + 1)
                nc.gpsimd.tensor_copy(dst[:, j, :, 0:HD + 1], src)

            def oproj_row(t):
                """partial out projection for 128-token row tile t."""
                ob = outp.tile([128, 1024], F32, name="ob", tag="ob")
                for nb in range(D // 512):
                    ps_o = psP.tile([128, 512], F32, name="psP", tag="psP")
                    for c in range(DL // 128):
                        nc.tensor.matmul(
                            ps_o,
                            yT_sb[c][:, 128 * t:128 * (t + 1)],
                            wp_all[:, c, 512 * nb:512 * (nb + 1)],
                            start=(c == 0), stop=(c == DL // 128 - 1),
                        )
                    nc.vector.tensor_copy(ob[:, 512 * nb:512 * (nb + 1)],
                                          ps_o)
                nc.sync.dma_start(
                    out=out[128 * t:128 * (t + 1), :],
                    in_=ob,
                )

            def attn_chunk(q0, c, filler):
                """causal attention for query block q0 (512 rows), head pair
                (2c, 2c+1).  Non-diagonal key tiles run PV as fp8 DoubleRow
                matmuls over tile pairs; the 4 diagonal tiles stay fp16.
                Pulls filler units between tiles to keep the PE fed while
                exps run."""
                ntiles = 4 * q0 + 4
                ps_ys = [psy.tile([128, 512], F32, name="psY",
                                  tag="psY") for p in range(2)]
                pv_backlog = []
                es8v = None

                def emit_pair_pv(u, v8):
                    for p in range(2):
                        h = 2 * c + p
                        w83 = w8[u].rearrange("p (j h m) -> p j h m",
                                              j=2, m=80)
                        nc.tensor.matmul(
                            ps_ys[p][0:HD + 1, 0:512],
                            w83[:, :, h, 0:HD + 1],
                            v8[:, :, 512 * p:512 * (p + 1)],
                            start=(u == 0), stop=False,
                            perf_mode=DR,
                        )

                def emit_diag_pv(t, lo, es):
                    for p in range(2):
                        h = 2 * c + p
                        nc.tensor.matmul(
                            ps_ys[p][0:HD + 1, lo:512],
                            vaug[t][:, (HD + 1) * h:(HD + 1) * (h + 1)],
                            es[:, 512 * p + lo:512 * (p + 1)],
                            start=(q0 == 0 and t == 0),
                            stop=(t == ntiles - 1),
                        )

                for t in range(ntiles):
                    m = t - 4 * q0
                    lo = 128 * m if m > 0 else 0
                    ps_s = psS.tile([128, 1024], F32, name="psS", tag="psS")
                    for p in range(2):  # two concurrent row-tiled matmuls
                        nc.tensor.matmul(
                            ps_s[:, 512 * p + lo:512 * (p + 1)],
                            kTp_sb[c][64 * p:64 * (p + 1),
                                      128 * t:128 * (t + 1)],
                            qT_sb[c][64 * p:64 * (p + 1),
                                     512 * q0 + lo:512 * (q0 + 1)],
                            start=True, stop=True,
                        )
                    if m < 0:
                        # non-diag: exp straight to the fp8 pair buffer
                        if t % 2 == 0:
                            es8 = work8.tile([128, 2048], F8, name="es8",
                                             tag="es8")
                            es8v = es8.rearrange("p (n u) -> p u n", u=2)
                        nc.scalar.activation(
                            out=es8v[:, t % 2, :],
                            in_=ps_s[:, 0:1024],
                            func=mybir.ActivationFunctionType.Exp,
                            scale=0.125 / 256.0,
                        )
                        if t % 2 == 1:
                            pv_backlog.append(
                                lambda u=t // 2, v8=es8v: emit_pair_pv(u, v8))
                    else:
                        es = work.tile([128, 1024], F16, name="es", tag="es")
                        es3 = es.rearrange("p (u f) -> p u f", u=2)
                        if lo == 0:
                            nc.scalar.activation(
                                out=es[:, 0:1024],
                                in_=ps_s[:, 0:1024],
                                func=mybir.ActivationFunctionType.Exp,
                                scale=0.125 / 256.0,
                            )
                        else:
                            ps3 = ps_s.rearrange("p (u f) -> p u f", u=2)
                            nc.scalar.activation(
                                out=es3[:, :, lo:512],
                                in_=ps3[:, :, lo:512],
                                func=mybir.ActivationFunctionType.Exp,
                                scale=0.125 / 256.0,
                            )
                        # zero the causally-masked region: es *= tri01
                        nc.vector.tensor_mul(
                            es3[:, :, lo:lo + 128],
                            es3[:, :, lo:lo + 128],
                            tri_sb.unsqueeze(1).broadcast_to([128, 2, 128]),
                        )
                        pv_backlog.append(
                            lambda t=t, lo=lo, es=es: emit_diag_pv(t, lo, es))
                    # pull a filler unit ahead of PV so the PE has
                    # independent work queued while this tile's exp runs
                    if filler and t % 5 == 4:
                        filler.pop(0)()
                    # keep the PV backlog ~2 units behind S
                    while len(pv_backlog) > 2:
                        pv_backlog.pop(0)()
                while pv_backlog:
                    pv_backlog.pop(0)()
                # bridge the chunk boundary: the normalize chain (copy ->
                # reciprocal -> broadcast -> multiply) takes ~3us before the
                # psy banks free for the next chunk's PV; give the PE real
                # work for that window (unmasked matmuls also keep the HAM
                # clock gate warm - masked S/PV matmuls don't register)
                if filler:
                    filler.pop(0)()
                for p in range(2):
                    poff = 64 * p
                    # tensor_copy rebases partition 64 -> 0 (the custom-DVE
                    # reciprocal op does not, so it must read from base 0)
                    dn = small.tile([1, 512], F32, name="dn", tag="dn")
                    nc.vector.tensor_copy(dn, ps_ys[p][HD:HD + 1, :])
                    rc1 = small.tile([1, 512], F32, name="rc1", tag="rc1")
                    nc.vector.reciprocal_approx_fast(rc1, dn)
                    rcb = small.tile([64, 512], F32, name="rcb", tag="rcb")
                    nc.gpsimd.partition_broadcast(rcb, rc1)
                    nc.vector.tensor_mul(
                        yT_sb[c][poff:poff + 64, 512 * q0:512 * (q0 + 1)],
                        ps_ys[p][0:HD, :],
                        rcb,
                    )

            # ---------- program ----------
            # stage 0: QKV+V for token block 0
            for cc in range(8):
                qkv_block(0, cc)
            for t in range(4):
                v_block(t)
                w8_fill(t)

            # stage 1: attn(q0=0) || QKV+V block 1
            f = [(lambda cc=cc: qkv_block(1, cc)) for cc in range(8)]
            f += [(lambda t=t: (v_block(t), w8_fill(t))) for t in range(4, 8)]
            for c in range(4):
                attn_chunk(0, c, f)
            for u in f:
                u()

            # stage 2: attn(q0=1) || QKV+V block 2
            f = [(lambda cc=cc: qkv_block(2, cc)) for cc in range(8)]
            f += [(lambda t=t: (v_block(t), w8_fill(t))) for t in range(8, 12)]
            for c in range(4):
                attn_chunk(1, c, f)
            for u in f:
                u()

            # stage 3: attn(q0=2) || QKV+V block 3
            f = [(lambda cc=cc: qkv_block(3, cc)) for cc in range(8)]
            f += [(lambda t=t: (v_block(t), w8_fill(t))) for t in range(12, 16)]
            for c in range(4):
                attn_chunk(2, c, f)
            for u in f:
                u()

            # stage 4: attn(q0=3) || oproj rows 0-11 (q0 blocks 0-2)
            f = [(lambda t=t: oproj_row(t)) for t in range(12)]
            for c in range(4):
                attn_chunk(3, c, f)
            for u in f:
                u()

            # stage 5: oproj rows 12-15 (q0 block 3)
            for t in range(12, 16):
                oproj_row(t)

    nc.finalize()
    return nc


def _enable_trace_hooks():
    """Inject antenv.axon_hooks + no-op artifact upload so that
    run_bass_kernel_spmd(trace=True) works under axon in this image."""
    import types
    import antenv

    if "antenv.axon_hooks" not in sys.modules:
        mod = types.ModuleType("antenv.axon_hooks")
        state = {"hook": None}
        mod.set_axon_ntff_profile_hook = lambda h: state.__setitem__("hook", h)
        mod.get_axon_ntff_profile_hook = lambda: state["hook"]
        sys.modules["antenv.axon_hooks"] = mod
        antenv.axon_hooks = mod
        from trn_agent_boot.trn_boot import _ntff_profile_via_ctypes

        mod.set_axon_ntff_profile_hook(
            _ntff_profile_via_ctypes("/opt/axon/libaxon_pjrt.so"))
    from concourse import bass_utils as bu

    bu.upload_artifacts = lambda tmpdir: str(tmpdir)


def kernel(x, w_attn, b_attn, w_proj, b_proj, _trace=False):
    x = np.asarray(x)
    w_attn = np.asarray(w_attn)
    b_attn = np.asarray(b_attn)
    w_proj = np.asarray(w_proj)
    b_proj = np.asarray(b_proj)

    if "nc" not in _cache:
        _cache["nc"] = _build()
    nc = _cache["nc"]

    f16 = np.float16
    f8 = ml_dtypes.float8_e4m3
    tri01 = np.where(np.arange(128)[:, None] <= np.arange(128)[None, :],
                     np.float16(1.0), np.float16(0.0)).astype(f16)

    def pairs_rhs(a):
        # [npair*2*128, cols] -> [128, npair, cols, 2] (j adjacent)
        npair = a.shape[0] // 256
        return np.ascontiguousarray(
            a.reshape(npair, 2, 128, a.shape[1]).transpose(2, 0, 3, 1)
        ).astype(f8)

    def pairs_lhs(a):
        # [npair*2*128, cols] -> [128, npair, 2, cols] (j-major)
        npair = a.shape[0] // 256
        return np.ascontiguousarray(
            a.reshape(npair, 2, 128, a.shape[1]).transpose(2, 0, 1, 3)
        ).astype(f8)

    in_maps = []
    for core in range(8):
        b, hg = core // 2, core % 2
        qs = slice(hg * DL, (hg + 1) * DL)
        ks = slice(D + hg * DL, D + (hg + 1) * DL)
        vs = slice(2 * D + hg * DL, 2 * D + (hg + 1) * DL)
        # the 1/sqrt(HD) score scale is folded into the exp's scale operand
        # (folding it into fp8 wq would push the weights into e4m3's
        # subnormal range)
        # qk weights are ~U(-1/32, 1/32): scale x16 so fp8 e4m3 quantizes
        # them in its normal range (min normal 2^-6) instead of subnormals;
        # the score path (16q).(16k) folds 1/256 into the exp scale.  The
        # value path (wv, wp) stays fp16: its quantization error is not
        # damped by softmax and would blow the error budget.
        wqk_host = 16.0 * np.concatenate([w_attn[:, qs], w_attn[:, ks]],
                                         axis=1).astype(np.float32)
        bq_host = 16.0 * b_attn[qs].astype(np.float32)
        xTb = x[b].T.astype(np.float32)

        def chunked(a, nch):
            return np.ascontiguousarray(
                a.reshape(nch, 128, a.shape[1]).transpose(1, 0, 2))

        in_maps.append({
            "xT8i": pairs_rhs(xTb),
            "xT": chunked(xTb.astype(f16), KCH),
            "w8qk": pairs_lhs(wqk_host),
            "bq": np.ascontiguousarray(bq_host.reshape(4, 128).T),
            "wv": chunked(w_attn[:, vs].astype(f16), KCH),
            "bv": np.ascontiguousarray(b_attn[vs][None, :]).astype(np.float32),
            "wp": chunked(w_proj[hg * DL:(hg + 1) * DL, :].astype(f16), 4),
            "tri": tri01,
        })

    kwargs = {}
    if _trace:
        _enable_trace_hooks()
        kwargs = dict(trace=True, trace_cores=[0])
    res = run_bass_kernel_spmd(nc, in_maps, core_ids=list(range(8)), **kwargs)

    outp = np.empty((B, T, D), np.float32)
    for b in range(B):
        outp[b] = res.results[2 * b]["out"] + res.results[2 * b + 1]["out"]
    outp += b_proj.astype(np.float32)

    if _trace:
        print(f"HW exec time: {res.exec_time_ns} ns")
    return outp
